# revision 5
# baseline (speedup 1.0000x reference)
"""MoE routing kernel for Trainium2: softmax over 256 experts + top-8 per token.

Full input: gating_output [131072, 256] f32. Output: (topk_weights f32,
topk_indices int32), both [131072, 8] — matching jax.lax.top_k semantics
(values descending, ties broken by lowest index first).

Strategy: shard tokens row-wise across 8 NeuronCores (16384 tokens each; the
computation is row-local so no communication). Per core, process 16 "big
tiles" of 1024 tokens laid out as [128 partitions x 8 subtiles x 256 experts]
with partition-contiguous 8 KiB DMA rows (token = (n*128 + p)*8 + t).

Per subtile [128, 256]:
  - DVE InstMax       -> top-8 raw logits per row, descending
  - DVE InstMaxIndex  -> their indices (duplicate values get ascending
                         distinct indices — matches jax.lax.top_k tie rules)
Per big tile:
  - ACT Exp over the whole [128, 2048] tile (no max-subtraction needed:
    |x| <= ~5.5 so exp is well within f32 range; softmax is shift-invariant)
  - Pool reduce_sum over [128, 8, 256] axis X -> per-row softmax denominators
  - DVE reciprocal + broadcast multiply -> weights = exp(top8) / sum(exp(row))

Top-8 selection runs on raw logits (softmax is monotone, so same selection),
which avoids f32 ties introduced by exp rounding.
"""

import numpy as np

TOKENS = 131072
EXPERTS = 256
K = 8
N_CORES = 8
TOK_PER_CORE = TOKENS // N_CORES  # 16384
P = 128
T_SUB = 8                  # subtiles (expert rows) per partition per big tile
BIG = P * T_SUB            # 1024 tokens per big tile
N_BIG = TOK_PER_CORE // BIG  # 16

_PROGRAM_CACHE = {}


def _build_program():
    import concourse.tile as tile
    from concourse import bacc, mybir

    f32 = mybir.dt.float32
    u32 = mybir.dt.uint32

    nc = bacc.Bacc("TRN2", debug=False, num_devices=N_CORES)

    g_dram = nc.dram_tensor(
        "gating", [TOK_PER_CORE, EXPERTS], f32, kind="ExternalInput"
    ).ap()
    w_dram = nc.dram_tensor(
        "weights", [TOK_PER_CORE, K], f32, kind="ExternalOutput"
    ).ap()
    i_dram = nc.dram_tensor(
        "indices", [TOK_PER_CORE, K], u32, kind="ExternalOutput"
    ).ap()

    # token = (n*P + p)*T_SUB + t: each partition reads T_SUB consecutive
    # 256-expert rows = one contiguous 8 KiB run per partition per big tile.
    g_t = g_dram.rearrange("(n p t) e -> n p (t e)", p=P, t=T_SUB)
    w_t = w_dram.rearrange("(n p t) k -> n p (t k)", p=P, t=T_SUB)
    i_t = i_dram.rearrange("(n p t) k -> n p (t k)", p=P, t=T_SUB)

    with tile.TileContext(nc) as tc:
        with (
            tc.tile_pool(name="gin", bufs=4) as gin_pool,
            tc.tile_pool(name="expbuf", bufs=2) as exp_pool,
            tc.tile_pool(name="outs", bufs=3) as out_pool,
        ):
            for n in range(N_BIG):
                gt = gin_pool.tile([P, T_SUB * EXPERTS], f32, name=f"gt{n}", tag="gt")
                nc.sync.dma_start(out=gt, in_=g_t[n])
                gt3 = gt.rearrange("p (t e) -> p t e", t=T_SUB)

                vals = out_pool.tile([P, T_SUB, K], f32, name=f"vals{n}", tag="vals")
                idxs = out_pool.tile([P, T_SUB, K], u32, name=f"idxs{n}", tag="idxs")
                sums = out_pool.tile([P, T_SUB], f32, name=f"sums{n}", tag="sums")
                for t in range(T_SUB):
                    nc.vector.max(out=vals[:, t, :], in_=gt3[:, t, :])
                    nc.vector.max_index(
                        out=idxs[:, t, :], in_max=vals[:, t, :], in_values=gt3[:, t, :]
                    )
                    # exp + per-row softmax denominator via the ACT accumulator
                    et = exp_pool.tile([P, EXPERTS], f32, name=f"et{n}_{t}", tag="et")
                    nc.scalar.activation(
                        out=et,
                        in_=gt3[:, t, :],
                        func=mybir.ActivationFunctionType.Exp,
                        accum_out=sums[:, t : t + 1],
                    )

                recips = out_pool.tile([P, T_SUB], f32, name=f"rec{n}", tag="rec")
                nc.vector.reciprocal(recips, sums)

                evals = out_pool.tile([P, T_SUB, K], f32, name=f"ev{n}", tag="ev")
                nc.scalar.activation(
                    out=evals,
                    in_=vals,
                    func=mybir.ActivationFunctionType.Exp,
                )
                wts = out_pool.tile([P, T_SUB, K], f32, name=f"wts{n}", tag="wts")
                nc.vector.tensor_mul(
                    wts,
                    evals,
                    recips.rearrange("p (t one) -> p t one", one=1).to_broadcast(
                        [P, T_SUB, K]
                    ),
                )

                nc.sync.dma_start(out=w_t[n], in_=wts.rearrange("p t k -> p (t k)"))
                nc.sync.dma_start(out=i_t[n], in_=idxs.rearrange("p t k -> p (t k)"))

    nc.compile()
    return nc


def kernel(**inputs) -> tuple:
    from concourse.bass_utils import run_bass_kernel_spmd

    gating = np.ascontiguousarray(np.asarray(inputs["gating_output"], dtype=np.float32))
    topk = int(np.asarray(inputs.get("topk", K)))
    assert topk == K, f"kernel hardcodes top-{K}, got topk={topk}"
    assert gating.shape == (TOKENS, EXPERTS), gating.shape

    if "nc" not in _PROGRAM_CACHE:
        _PROGRAM_CACHE["nc"] = _build_program()
    nc = _PROGRAM_CACHE["nc"]

    shards = gating.reshape(N_CORES, TOK_PER_CORE, EXPERTS)
    in_maps = [{"gating": shards[c]} for c in range(N_CORES)]
    res = run_bass_kernel_spmd(nc, in_maps, core_ids=list(range(N_CORES)))
    _PROGRAM_CACHE["last_results"] = res

    weights = np.concatenate([r["weights"] for r in res.results], axis=0)
    indices = np.concatenate([r["indices"] for r in res.results], axis=0)
    return weights.astype(np.float32, copy=False), indices.astype(np.int32, copy=False)
